# revision 4
# baseline (speedup 1.0000x reference)
"""Trainium2 Bass kernel for nn_DifferentiableProjector (volume rendering), v11.

Math (per ray i, samples s=0..S-1, channels c):
    T_excl[s] = exp(-DT * sum_{s'<s} rho[s'])
    T_incl[s] = exp(-DT * sum_{s'<=s} rho[s'])
    w[s]      = T_excl[s] - T_incl[s]
    out[i,c]  = sum_s w[s] * f[i,s,c]

v6 = fp16 pipeline with tightened DMA orchestration (the hybrid int8 SWDGE
cast-stream experiments v2-v5 lost more to Q7 serial descriptor emission
+ startup serialization than they saved in HBM traffic):
  - host pretransposes rho to [s, (tile, i)] so its DMA rows are 16KB
    contiguous (the old layout generated 1KB descriptor lines and delayed
    the first matmul by ~20-35us)
  - sync engine issues ONLY f tile loads (two 1MB chunks per tile; first
    tile in four chunks so the first multiply starts ~5us earlier);
    scalar issues consts/rho-rest/output stores
  - fp16 output (dequant/cast on host) halves evac + store traffic
  - last two tiles are reduced singly to shorten the tail

Sharding: data-parallel over rays, 65536 rays -> 8 cores x 8192 rays.
"""

import numpy as np

import concourse.bass as bass
import concourse.tile as tile
from concourse.bacc import Bacc
from concourse import mybir
from concourse.bass_utils import run_bass_kernel_spmd

H, W, S, C = 256, 256, 128, 16
N = H * W
NCORES = 8
NS = N // NCORES          # rays per core
P = 128                   # partitions (= S)
T = 512                   # rays per tile
NTILES = NS // T
DT = (6.0 - 2.0) / S

_cached = {}

TRACE = False
LAST_RESULTS = None

F16 = mybir.dt.float16
F32 = mybir.dt.float32


def _build_nc(ns: int = NS) -> bass.Bass:
    ntiles = ns // T
    nc = Bacc()
    rho_d = nc.dram_tensor("rho", [S, ntiles * T], F16, kind="ExternalInput")
    f_d = nc.dram_tensor("f", [ntiles * S, C * T], F16, kind="ExternalInput")
    cst_d = nc.dram_tensor("consts", [P, 2 * P + C * C], F16, kind="ExternalInput")
    out_d = nc.dram_tensor("out", [C, ns], F16, kind="ExternalOutput")

    with tile.TileContext(nc) as tc:
        with (
            tc.tile_pool(name="cpool", bufs=1) as cpool,
            tc.tile_pool(name="fpool", bufs=6) as fpool,
            tc.tile_pool(name="tpool", bufs=3) as tpool,
            tc.tile_pool(name="spool", bufs=3) as spool,
            tc.tile_pool(name="opool", bufs=3) as opool,
            tc.tile_pool(name="psc", bufs=2, space="PSUM") as psc,
            tc.tile_pool(name="pso", bufs=2, space="PSUM") as pso,
        ):
            consts = cpool.tile([P, 2 * P + C * C], F16)
            nc.scalar.dma_start(out=consts, in_=cst_d[:, :])
            u_excl = consts[:, 0:P]
            u_incl = consts[:, P : 2 * P]
            e_base = 2 * P

            # rho slabs, host-pretransposed to [s, (tile, i)]: contiguous
            # 16KB DRAM rows. Tiles 0-1 first on sync (unblocks the first
            # cumsum fast), rest on scalar.
            rho_all = cpool.tile([P, ntiles, T], F16)
            rho_src = rho_d[:, :].rearrange("s (t i) -> s t i", t=ntiles)
            nc.sync.dma_start(out=rho_all[:, 0:2, :], in_=rho_src[:, 0:2, :])
            nc.scalar.dma_start(
                out=rho_all[:, 2:ntiles, :], in_=rho_src[:, 2:ntiles, :]
            )

            def tile_front(t):
                """DMA load + w pipeline + multiply for tile t."""
                fT = fpool.tile([P, C, T], F16, tag="fT")
                f_slab = f_d[t * S : (t + 1) * S, :].rearrange(
                    "s (c i) -> s c i", c=C
                )
                nchunk = 4 if t == 0 else 1
                for q in range(nchunk):
                    c0 = q * (C // nchunk)
                    nc.sync.dma_start(
                        out=fT[:, c0 : c0 + C // nchunk, :],
                        in_=f_slab[:, c0 : c0 + C // nchunk, :],
                    )
                rhoT = rho_all[:, t, :]

                pexc = psc.tile([P, T], F32, tag="pexc")
                pinc = psc.tile([P, T], F32, tag="pinc")
                nc.tensor.matmul(pexc, u_excl, rhoT, start=True, stop=True)
                nc.tensor.matmul(pinc, u_incl, rhoT, start=True, stop=True)

                # exps in fp32 (w = e1 - e2 cancels; fp16 would cost ~4% on w)
                e1 = spool.tile([P, T], F32, tag="e1")
                e2 = spool.tile([P, T], F32, tag="e2")
                nc.scalar.activation(
                    e1, pexc, mybir.ActivationFunctionType.Exp, scale=-DT
                )
                nc.scalar.activation(
                    e2, pinc, mybir.ActivationFunctionType.Exp, scale=-DT
                )
                w = spool.tile([P, T], F16, tag="w")
                nc.vector.tensor_sub(w, e1, e2)

                # tmp[s, c, i] = fT[s, c, i] * w[s, i], quarter chunks
                tmp = tpool.tile([P, C, T], F16, tag="tmp")
                nsplit = 4
                for q in range(nsplit):
                    c0 = q * (C // nsplit)
                    nc.vector.tensor_mul(
                        tmp[:, c0 : c0 + C // nsplit, :],
                        fT[:, c0 : c0 + C // nsplit, :],
                        w[:, None, :].broadcast_to((P, C // nsplit, T)),
                    )
                return tmp

            def tile_back(t, tmps):
                """Reduce + evacuate tiles t..t+len(tmps)-1, sharing each
                E_c weight load across the group."""
                k = len(tmps)
                psums = [
                    pso.tile([C, T], F32, tag=f"po{i}", name=f"po{i}_{t}")
                    for i in range(k)
                ]
                for c in range(C):
                    lhs = consts[:, e_base + c * C : e_base + (c + 1) * C]
                    for tmp_t, po in zip(tmps, psums):
                        nc.tensor.matmul(
                            po,
                            lhs,
                            tmp_t[:, c, :],
                            start=(c == 0),
                            stop=(c == C - 1),
                        )
                out_g = opool.tile(
                    [C, k * T], F16, tag="out_g", name=f"out_g_{t}"
                )
                for j, po in enumerate(psums):
                    nc.scalar.activation(
                        out_g[:, j * T : (j + 1) * T],
                        po,
                        mybir.ActivationFunctionType.Copy,
                    )
                nc.scalar.dma_start(
                    out=out_d[:, t * T : (t + k) * T],
                    in_=out_g,
                )

            for t in range(0, ntiles - 2, 2):
                tmp_a = tile_front(t)
                tmp_b = tile_front(t + 1)
                tile_back(t, [tmp_a, tmp_b])
            # last pair: per-tile reduce to shorten the tail
            tmp_a = tile_front(ntiles - 2)
            tile_back(ntiles - 2, [tmp_a])
            tmp_b = tile_front(ntiles - 1)
            tile_back(ntiles - 1, [tmp_b])
    if not nc.is_finalized():
        nc.finalize()
    return nc


def _consts() -> np.ndarray:
    u_excl = np.triu(np.ones((P, P), np.float16), 1)
    u_incl = np.triu(np.ones((P, P), np.float16), 0)
    e = np.tile(np.eye(C, dtype=np.float16).reshape(1, C * C), (P, 1))
    return np.ascontiguousarray(np.concatenate([u_excl, u_incl, e], axis=1))


def kernel(rho: np.ndarray, f: np.ndarray) -> np.ndarray:
    global LAST_RESULTS
    if "nc" not in _cached:
        _cached["nc"] = _build_nc()
        _cached["consts"] = _consts()
    nc = _cached["nc"]

    rho16 = np.asarray(rho, dtype=np.float16).reshape(N, S)
    f16 = np.asarray(f, dtype=np.float16).reshape(N, S, C)
    cst = _cached["consts"]
    ntiles = NTILES

    in_maps = []
    for i in range(NCORES):
        sl = slice(i * NS, (i + 1) * NS)
        rho_t = np.ascontiguousarray(
            rho16[sl].reshape(ntiles, T, S).transpose(2, 0, 1)
        ).reshape(S, ntiles * T)
        f_t = np.ascontiguousarray(
            f16[sl].reshape(ntiles, T, S, C).transpose(0, 2, 3, 1)
        ).reshape(ntiles * S, C * T)
        in_maps.append({"rho": rho_t, "f": f_t, "consts": cst})
    res = run_bass_kernel_spmd(nc, in_maps, list(range(NCORES)), trace=TRACE)
    LAST_RESULTS = res
    out = np.concatenate(
        [res.results[i]["out"] for i in range(NCORES)], axis=1
    )
    return out.reshape(C, H, W)[None].astype(np.float32, copy=False)


# revision 5
# speedup vs baseline: 1.0081x; 1.0081x over previous
"""Trainium2 Bass kernel for nn_DifferentiableProjector (volume rendering), v12.

Math (per ray i, samples s=0..S-1, channels c):
    T_excl[s] = exp(-DT * sum_{s'<s} rho[s'])
    T_incl[s] = exp(-DT * sum_{s'<=s} rho[s'])
    w[s]      = T_excl[s] - T_incl[s]
    out[i,c]  = sum_s w[s] * f[i,s,c]

v6 = fp16 pipeline with tightened DMA orchestration (the hybrid int8 SWDGE
cast-stream experiments v2-v5 lost more to Q7 serial descriptor emission
+ startup serialization than they saved in HBM traffic):
  - host pretransposes rho to [s, (tile, i)] so its DMA rows are 16KB
    contiguous (the old layout generated 1KB descriptor lines and delayed
    the first matmul by ~20-35us)
  - sync engine issues ONLY f tile loads (two 1MB chunks per tile; first
    tile in four chunks so the first multiply starts ~5us earlier);
    scalar issues consts/rho-rest/output stores
  - fp16 output (dequant/cast on host) halves evac + store traffic
  - last two tiles are reduced singly to shorten the tail

Sharding: data-parallel over rays, 65536 rays -> 8 cores x 8192 rays.
"""

import numpy as np

import concourse.bass as bass
import concourse.tile as tile
from concourse.bacc import Bacc
from concourse import mybir
from concourse.bass_utils import run_bass_kernel_spmd

H, W, S, C = 256, 256, 128, 16
N = H * W
NCORES = 8
NS = N // NCORES          # rays per core
P = 128                   # partitions (= S)
T = 512                   # rays per tile
NTILES = NS // T
DT = (6.0 - 2.0) / S

_cached = {}

TRACE = False
LAST_RESULTS = None

F16 = mybir.dt.float16
F32 = mybir.dt.float32


def _build_nc(ns: int = NS) -> bass.Bass:
    ntiles = ns // T
    nc = Bacc()
    rho_d = nc.dram_tensor("rho", [S, ntiles * T], F16, kind="ExternalInput")
    f_d = nc.dram_tensor("f", [ntiles * S, C * T], F16, kind="ExternalInput")
    cst_d = nc.dram_tensor("consts", [P, 2 * P + C * C], F16, kind="ExternalInput")
    out_d = nc.dram_tensor("out", [C, ns], F16, kind="ExternalOutput")

    with tile.TileContext(nc) as tc:
        with (
            tc.tile_pool(name="cpool", bufs=1) as cpool,
            tc.tile_pool(name="fpool", bufs=6) as fpool,
            tc.tile_pool(name="tpool", bufs=4) as tpool,
            tc.tile_pool(name="spool", bufs=3) as spool,
            tc.tile_pool(name="opool", bufs=3) as opool,
            tc.tile_pool(name="psc", bufs=2, space="PSUM") as psc,
            tc.tile_pool(name="pso", bufs=2, space="PSUM") as pso,
        ):
            consts = cpool.tile([P, 2 * P + C * C], F16)
            nc.scalar.dma_start(out=consts, in_=cst_d[:, :])
            u_excl = consts[:, 0:P]
            u_incl = consts[:, P : 2 * P]
            e_base = 2 * P

            # rho slabs, host-pretransposed to [s, (tile, i)]: contiguous
            # 16KB DRAM rows. Tiles 0-1 first on sync (unblocks the first
            # cumsum fast), rest on scalar.
            rho_all = cpool.tile([P, ntiles, T], F16)
            rho_src = rho_d[:, :].rearrange("s (t i) -> s t i", t=ntiles)
            nc.sync.dma_start(out=rho_all[:, 0:2, :], in_=rho_src[:, 0:2, :])
            nc.scalar.dma_start(
                out=rho_all[:, 2:ntiles, :], in_=rho_src[:, 2:ntiles, :]
            )

            def tile_front(t):
                """DMA load + w pipeline + multiply for tile t."""
                fT = fpool.tile([P, C, T], F16, tag="fT")
                f_slab = f_d[t * S : (t + 1) * S, :].rearrange(
                    "s (c i) -> s c i", c=C
                )
                nchunk = 4 if t == 0 else 1
                for q in range(nchunk):
                    c0 = q * (C // nchunk)
                    nc.sync.dma_start(
                        out=fT[:, c0 : c0 + C // nchunk, :],
                        in_=f_slab[:, c0 : c0 + C // nchunk, :],
                    )
                rhoT = rho_all[:, t, :]

                pexc = psc.tile([P, T], F32, tag="pexc")
                pinc = psc.tile([P, T], F32, tag="pinc")
                nc.tensor.matmul(pexc, u_excl, rhoT, start=True, stop=True)
                nc.tensor.matmul(pinc, u_incl, rhoT, start=True, stop=True)

                # exps in fp32 (w = e1 - e2 cancels; fp16 would cost ~4% on w)
                e1 = spool.tile([P, T], F32, tag="e1")
                e2 = spool.tile([P, T], F32, tag="e2")
                nc.scalar.activation(
                    e1, pexc, mybir.ActivationFunctionType.Exp, scale=-DT
                )
                nc.scalar.activation(
                    e2, pinc, mybir.ActivationFunctionType.Exp, scale=-DT
                )
                w = spool.tile([P, T], F16, tag="w")
                nc.vector.tensor_sub(w, e1, e2)

                # tmp[s, c, i] = fT[s, c, i] * w[s, i], quarter chunks
                tmp = tpool.tile([P, C, T], F16, tag="tmp")
                nsplit = 4
                for q in range(nsplit):
                    c0 = q * (C // nsplit)
                    nc.vector.tensor_mul(
                        tmp[:, c0 : c0 + C // nsplit, :],
                        fT[:, c0 : c0 + C // nsplit, :],
                        w[:, None, :].broadcast_to((P, C // nsplit, T)),
                    )
                return tmp

            def tile_back(t, tmps):
                """Reduce + evacuate tiles t..t+len(tmps)-1, sharing each
                E_c weight load across the group."""
                k = len(tmps)
                psums = [
                    pso.tile([C, T], F32, tag=f"po{i}", name=f"po{i}_{t}")
                    for i in range(k)
                ]
                for c in range(C):
                    lhs = consts[:, e_base + c * C : e_base + (c + 1) * C]
                    for tmp_t, po in zip(tmps, psums):
                        nc.tensor.matmul(
                            po,
                            lhs,
                            tmp_t[:, c, :],
                            start=(c == 0),
                            stop=(c == C - 1),
                        )
                out_g = opool.tile(
                    [C, k * T], F16, tag="out_g", name=f"out_g_{t}"
                )
                for j, po in enumerate(psums):
                    nc.scalar.activation(
                        out_g[:, j * T : (j + 1) * T],
                        po,
                        mybir.ActivationFunctionType.Copy,
                    )
                nc.scalar.dma_start(
                    out=out_d[:, t * T : (t + k) * T],
                    in_=out_g,
                )

            for t in range(0, ntiles - 2, 2):
                tmp_a = tile_front(t)
                tmp_b = tile_front(t + 1)
                tile_back(t, [tmp_a, tmp_b])
            # last pair: per-tile reduce to shorten the tail
            tmp_a = tile_front(ntiles - 2)
            tile_back(ntiles - 2, [tmp_a])
            tmp_b = tile_front(ntiles - 1)
            tile_back(ntiles - 1, [tmp_b])
    if not nc.is_finalized():
        nc.finalize()
    return nc


def _consts() -> np.ndarray:
    u_excl = np.triu(np.ones((P, P), np.float16), 1)
    u_incl = np.triu(np.ones((P, P), np.float16), 0)
    e = np.tile(np.eye(C, dtype=np.float16).reshape(1, C * C), (P, 1))
    return np.ascontiguousarray(np.concatenate([u_excl, u_incl, e], axis=1))


def kernel(rho: np.ndarray, f: np.ndarray) -> np.ndarray:
    global LAST_RESULTS
    if "nc" not in _cached:
        _cached["nc"] = _build_nc()
        _cached["consts"] = _consts()
    nc = _cached["nc"]

    rho16 = np.asarray(rho, dtype=np.float16).reshape(N, S)
    f16 = np.asarray(f, dtype=np.float16).reshape(N, S, C)
    cst = _cached["consts"]
    ntiles = NTILES

    in_maps = []
    for i in range(NCORES):
        sl = slice(i * NS, (i + 1) * NS)
        rho_t = np.ascontiguousarray(
            rho16[sl].reshape(ntiles, T, S).transpose(2, 0, 1)
        ).reshape(S, ntiles * T)
        f_t = np.ascontiguousarray(
            f16[sl].reshape(ntiles, T, S, C).transpose(0, 2, 3, 1)
        ).reshape(ntiles * S, C * T)
        in_maps.append({"rho": rho_t, "f": f_t, "consts": cst})
    res = run_bass_kernel_spmd(nc, in_maps, list(range(NCORES)), trace=TRACE)
    LAST_RESULTS = res
    out = np.concatenate(
        [res.results[i]["out"] for i in range(NCORES)], axis=1
    )
    return out.reshape(C, H, W)[None].astype(np.float32, copy=False)
